# revision 4
# baseline (speedup 1.0000x reference)
"""Trainium2 Bass kernel for nn_BinaryTreeLogicNet — transposed tree, v12.

v5 over v4:
  - all x loads prefetched up front on sync (2KB-contiguous runs per call)
  - repack DMAs: A-half on gpsimd, B-half on sync, both in program order
    (kills head-of-line blocking between x loads / repacks / outputs)
  - scs 0..2: DMA-repacked tails (cheap on DVE, hops hidden); sc3 (last):
    RPT=2 + all-DVE copy/shuffle tail for the shortest exposed chain
"""

import numpy as np
import ml_dtypes

import concourse.bass as bass
import concourse.bacc as bacc
import concourse.mybir as mybir
import concourse.tile as tile
from concourse.bass_utils import run_bass_kernel_spmd

import concourse.dve_ops as dve_ops
from concourse.dve_spec import Spec, Src0, Src1, C0, C1, minn, lower, _has_src1
from concourse.dve_uop import DveOpSpec


def _gcd2_ref(in0, in1, s0, s1, imm2):
    a = in0.astype(np.float32)
    b = in1.astype(np.float32) * np.asarray(s0, np.float32).reshape(-1, 1)
    c = np.asarray(s1, np.float32).reshape(-1, 1)
    return (a + b) + np.minimum(a * c, b * c)


def _register_gcd2():
    name = "GCD_NODE2_ANT"
    if name in dve_ops._SUB_OPCODE_FOR_NAME:
        return next(op for op in dve_ops.OPS if op.name == name)
    sr = Src1 * C0
    spec = Spec(body=(Src0 + sr) + minn(Src0 * C1, sr * C1), reference=_gcd2_ref)
    row = 1 + len(dve_ops.OPS)
    uops = lower(spec, ver="v3")
    sha = DveOpSpec(name=name, opcode=row, uops=uops, rd1_en=_has_src1(spec)).sha("v3")
    op = dve_ops.DveOp(name, spec, subdim=False, uops_sha={"v3": sha})
    dve_ops.OPS.append(op)
    dve_ops._SUB_OPCODE_FOR_NAME[name] = row
    dve_ops.CUSTOM_DVE_SPECS[name] = spec
    return op


GCD2 = _register_gcd2()

B, L = 65536, 256
N_CORES = 8
BS = B // N_CORES        # 8192
NSC = 4
SCB = BS // NSC          # 2048
RPT = 4                  # repack depth for scs 0..2 (then all-DVE tail)
RPT_LAST = 3             # repack depth for the standalone sc (B-halves on gpsimd)
BIAS_SHIFT = -2.0

MM_DT = mybir.dt.float8e4
MM_NP = ml_dtypes.float8_e4m3
TREE_DT = mybir.dt.float16


def _sigmoid(z):
    return 1.0 / (1.0 + np.exp(-z))


def _bitrev(n):
    bits = n.bit_length() - 1
    out = np.zeros(n, np.int64)
    for j in range(n):
        r, x = 0, j
        for _ in range(bits):
            r = (r << 1) | (x & 1)
            x >>= 1
        out[j] = r
    return out


def prep_tree_consts(weights, biases, w_out):
    w = weights.astype(np.float64)
    b = biases.astype(np.float64)
    g = np.ones(256, np.float64)
    alphas, chats = [], []
    off = 0
    for t in range(8):
        m = 128 >> t
        jref = _bitrev(m) if m > 1 else np.array([0])
        lam = _sigmoid(b[off + jref])
        w1 = w[off + jref, 0]
        w2 = w[off + jref, 1]
        alphas.append((w2 * g[m : 2 * m]) / (w1 * g[:m]))
        chats.append((1.0 - 2.0 * lam) / lam)
        g = lam * w1 * g[:m]
        off += m
    gscale = float(g[0] * w_out[0, 0])
    return alphas, chats, gscale


def _sim_repack(idx):
    P, F = idx.shape
    out = np.empty((P, F // 2), idx.dtype)
    for p in range(P // 2):
        for bb in range(2):
            out[2 * p + bb] = idx[p, bb * (F // 2) : (bb + 1) * (F // 2)]
    return out


def _perm_maps():
    # scs 0..2: RPT repacks then partition halvings to P = 128 >> (7-RPT)
    idx = np.broadcast_to(np.arange(SCB), (128, SCB)).copy()
    for _ in range(RPT):
        idx = _sim_repack(idx)
    map_rp = idx[: (128 >> (7 - RPT)), :]
    # last sc: RPT_LAST repacks then partition halvings to P = 128>>5 = 4
    idx = np.broadcast_to(np.arange(SCB), (128, SCB)).copy()
    for _ in range(RPT_LAST):
        idx = _sim_repack(idx)
    map_cs = idx[: (128 >> (7 - RPT_LAST)), :]
    return map_rp, map_cs


# ---------------- bass program ----------------

def build_nc(gscale):
    nc = bacc.Bacc("TRN2", target_bir_lowering=False, debug=False)
    f32 = mybir.dt.float32

    xt = nc.dram_tensor("xt", [2, 128, BS], MM_DT, kind="ExternalInput")
    wts = nc.dram_tensor("wts", [128, 2, 2, 128], MM_DT, kind="ExternalInput")
    cst = nc.dram_tensor("cst", [128, 36], f32, kind="ExternalInput")
    PR = 128 >> (7 - RPT)   # 16
    FR = SCB >> RPT         # 128
    outr = nc.dram_tensor("outr", [PR, NSC - 1, FR], f32, kind="ExternalOutput")
    PL = 128 >> (7 - RPT_LAST)  # 4
    FL = SCB >> RPT_LAST        # 512
    outc = nc.dram_tensor("outc", [PL, FL], f32, kind="ExternalOutput")

    # lane p <- lane (p + P//2) shuffles for partition halving below 64
    def shmask(P):
        m = list(range(32))
        for p in range(P // 2):
            m[p] = p + P // 2
        return m

    with tile.TileContext(nc) as tc:
        with (
            tc.tile_pool(name="const", bufs=1) as constp,
            tc.tile_pool(name="leaf", bufs=2) as leafp,
            tc.tile_pool(name="work", bufs=2) as workp,
            tc.tile_pool(name="psum", bufs=1, space="PSUM") as psp,
        ):
            cst_sb = constp.tile([128, 36], f32)
            nc.scalar.dma_start(out=cst_sb[:, :], in_=cst.ap())
            wts_sb = constp.tile([128, 2, 2, 128], MM_DT)
            nc.scalar.dma_start(out=wts_sb[:, :, :, :], in_=wts.ap())

            # prefetch ALL x up front (sync queue, no later blockers)
            xsbs = {}
            for sc in [NSC - 1] + list(range(NSC - 1)):
                xsb = constp.tile([128, 2, SCB], MM_DT, tag=f"xsb{sc}")
                if sc == NSC - 1:
                    # fine pieces, kc-interleaved: matmul c can start after
                    # its two 512-col pieces arrive
                    for q in range(SCB // 512):
                        for kc in range(2):
                            nc.sync.dma_start(
                                out=xsb[:, kc, q * 512 : (q + 1) * 512],
                                in_=xt.ap()[kc, :, sc * SCB + q * 512 : sc * SCB + (q + 1) * 512],
                            )
                else:
                    for kc in range(2):
                        nc.sync.dma_start(
                            out=xsb[:, kc, :],
                            in_=xt.ap()[kc, :, sc * SCB : (sc + 1) * SCB],
                        )
                xsbs[sc] = xsb

            def alpha(t, P=128):
                return cst_sb[0:P, t : t + 1]

            def chat(t, P=128):
                return cst_sb[0:P, 8 + t : 9 + t]

            bias_shift = cst_sb[:, 16:17]
            bias_out = cst_sb[:, 17:18]

            def alpha_cs(t, P):  # last-sc copy-shift consts (t = RPT_LAST+1..7)
                return cst_sb[0:P, 26 + (t - RPT_LAST - 1) : 27 + (t - RPT_LAST - 1)]

            def chat_cs(t, P):
                return cst_sb[0:P, 31 + (t - RPT_LAST - 1) : 32 + (t - RPT_LAST - 1)]

            wu = constp.tile([128, 8], TREE_DT)
            nc.vector.memset(wu[:, :], 1.0)
            nc.vector._custom_dve(
                GCD2, out=wu[:, 0:4], in0=wu[:, 4:8], in1=wu[:, 4:8], s0=1.0, s1=-0.5
            )

            def levelop(t, in0, in1, out, P=128):
                nc.vector._custom_dve(
                    GCD2, out=out, in0=in0, in1=in1,
                    s0=alpha(t, P), s1=chat(t, P),
                )

            FR4 = SCB >> RPT  # 128
            FS = (NSC - 1) * FR4  # 384
            n4s = workp.tile([128, FS], TREE_DT, tag="n4s", bufs=1)

            def repack(cur, F, t, beng=None):
                ab = workp.tile([128, 2, F // 2], TREE_DT, tag=f"ab{t}")
                nc.gpsimd.dma_start(
                    out=ab[:, 0, :],
                    in_=cur[0:64, :].rearrange("p (b f) -> p b f", b=2),
                )
                (beng or nc.sync).dma_start(
                    out=ab[:, 1, :],
                    in_=cur[64:128, :].rearrange("p (b f) -> p b f", b=2),
                )
                return ab

            for sc in [NSC - 1] + list(range(NSC - 1)):
                last = sc == NSC - 1
                xsb = xsbs[sc]
                leafA = leafp.tile([128, SCB], TREE_DT, tag="lfA")
                leafB = leafp.tile([128, SCB], TREE_DT, tag="lfB")
                for half, lf in ((0, leafA), (1, leafB)):
                    ps = psp.tile([128, SCB], mybir.dt.float32, tag=f"ps{half}")
                    lhsT = wts_sb[:, half, :, :]
                    for c in range(SCB // 512):
                        nc.tensor.matmul(
                            ps[:, c * 512 : (c + 1) * 512],
                            lhsT,
                            xsb[:, :, c * 512 : (c + 1) * 512],
                            start=True,
                            stop=True,
                            perf_mode=mybir.MatmulPerfMode.DoubleRow,
                        )
                    nc.scalar.activation(
                        out=lf[:, :],
                        in_=ps[:, :],
                        func=mybir.ActivationFunctionType.Sigmoid,
                        bias=bias_shift,
                        scale=1.0,
                    )

                n0 = workp.tile([128, SCB], TREE_DT, tag="n0")
                levelop(0, leafA[:, :], leafB[:, :], n0[:, :])
                cur, F = n0, SCB
                rpt = RPT_LAST if last else RPT
                for t in range(1, rpt + 1):
                    ab = repack(cur, F, t, beng=nc.gpsimd if last else None)
                    if not last and t == rpt:
                        levelop(t, ab[:, 0, :], ab[:, 1, :], n4s[:, sc * FR4 : (sc + 1) * FR4])
                        cur, F = None, F // 2
                        break
                    nxt = workp.tile([128, F // 2], TREE_DT, tag=f"n{t}")
                    levelop(t, ab[:, 0, :], ab[:, 1, :], nxt[:, :])
                    cur, F = nxt, F // 2

                if not last:
                    pass  # levels RPT+1..7 run once in the shared tail below
                else:
                    P = 128
                    for t in range(RPT_LAST + 1, 8):
                        if (P // 2) % 32 == 0:
                            half = workp.tile([P // 2, F], TREE_DT, tag=f"h{t}")
                            nc.vector.tensor_copy(
                                out=half[:, :], in_=cur[P // 2 : P, :]
                            )
                            h = half[:, :]
                        else:
                            half = workp.tile([P, F], TREE_DT, tag=f"h{t}")
                            nc.vector.stream_shuffle(
                                out=half[:, :], in_=cur[:, :], mask=shmask(P)
                            )
                            h = half[0 : P // 2, :]
                        nxt = workp.tile([P // 2, F], TREE_DT, tag=f"t{t}")
                        nc.vector._custom_dve(
                            GCD2,
                            out=nxt[:, :],
                            in0=cur[0 : P // 2, :],
                            in1=h,
                            s0=alpha_cs(t, P // 2),
                            s1=chat_cs(t, P // 2),
                        )
                        cur, P = nxt, P // 2
                    final = workp.tile([PL, FL], f32, tag="finc")
                    nc.scalar.activation(
                        out=final[:, :],
                        in_=cur[:, :],
                        func=mybir.ActivationFunctionType.Sigmoid,
                        bias=bias_out[0:PL, :],
                        scale=float(gscale),
                    )
                    nc.sync.dma_start(out=outc.ap(), in_=final[:, :])

            # shared copy/shuffle tail over scs 0..NSC-2
            curT, P = n4s, 128
            for t in range(RPT + 1, 8):
                if (P // 2) % 32 == 0:
                    half = workp.tile([P // 2, FS], TREE_DT, tag=f"sh{t}")
                    nc.vector.tensor_copy(
                        out=half[:, :], in_=curT[P // 2 : P, :]
                    )
                    h = half[:, :]
                else:
                    half = workp.tile([P, FS], TREE_DT, tag=f"sh{t}")
                    nc.vector.stream_shuffle(
                        out=half[:, :], in_=curT[:, :], mask=shmask(P)
                    )
                    h = half[0 : P // 2, :]
                nxt = workp.tile([P // 2, FS], TREE_DT, tag=f"st{t}")
                nc.vector._custom_dve(
                    GCD2, out=nxt[:, :], in0=curT[0 : P // 2, :], in1=h,
                    s0=cst_sb[0 : P // 2, 18 + (t - RPT - 1) : 19 + (t - RPT - 1)],
                    s1=cst_sb[0 : P // 2, 21 + (t - RPT - 1) : 22 + (t - RPT - 1)],
                )
                curT, P = nxt, P // 2
            finalS = workp.tile([P, FS], f32, tag="finS")
            nc.scalar.activation(
                out=finalS[:, :],
                in_=curT[:, :],
                func=mybir.ActivationFunctionType.Sigmoid,
                bias=bias_out[0:P, :],
                scale=float(gscale),
            )
            nc.sync.dma_start(out=outr.ap(), in_=finalS[:, :])

    nc.compile()
    return nc


# ---------------- host side ----------------

def make_in_maps(x, W_leaf, weights, biases, w_out, b_out):
    alphas, chats, gscale = prep_tree_consts(weights, biases, w_out)
    cstv = np.zeros((128, 36), np.float32)
    for t in range(8):
        col_a = np.repeat(alphas[t], 1 << t).astype(np.float32)
        col_c = np.repeat(chats[t], 1 << t).astype(np.float32)
        cstv[:, t] = col_a[:128]
        cstv[:, 8 + t] = col_c[:128]
    for t in range(RPT_LAST + 1, 8):
        col_a = np.repeat(alphas[t], 1 << RPT_LAST).astype(np.float32)
        col_c = np.repeat(chats[t], 1 << RPT_LAST).astype(np.float32)
        cstv[: len(col_a), 26 + (t - RPT_LAST - 1)] = col_a
        cstv[: len(col_c), 31 + (t - RPT_LAST - 1)] = col_c
    for t in range(RPT + 1, 8):
        col_a = np.repeat(alphas[t], 1 << RPT).astype(np.float32)[:128]
        col_c = np.repeat(chats[t], 1 << RPT).astype(np.float32)[:128]
        cstv[: len(col_a), 18 + (t - RPT - 1)] = col_a
        cstv[: len(col_c), 21 + (t - RPT - 1)] = col_c
    cstv[:, 16] = BIAS_SHIFT
    cstv[:, 17] = float(b_out[0])

    W_perm = W_leaf[_bitrev(256)]
    WT = W_perm.T.astype(np.float32)
    wts_host = np.empty((128, 2, 2, 128), np.float32)
    for half in range(2):
        for i in range(2):
            wts_host[:, half, i, :] = WT[i * 128 : (i + 1) * 128,
                                         half * 128 : (half + 1) * 128]
    wts_host = np.ascontiguousarray(wts_host.astype(MM_NP))

    xT = np.ascontiguousarray(x.T.astype(MM_NP).reshape(2, 128, B))
    in_maps = []
    for c in range(N_CORES):
        sh = np.ascontiguousarray(xT[:, :, c * BS : (c + 1) * BS])
        in_maps.append({"xt": sh, "wts": wts_host, "cst": cstv})
    return in_maps, gscale


_MAPS = None


def gather_out(results):
    global _MAPS
    if _MAPS is None:
        _MAPS = _perm_maps()
    map_rp, map_cs = _MAPS
    full = np.empty((B, 1), np.float32)
    for c in range(N_CORES):
        rr = np.asarray(results[c]["outr"], np.float32)
        for sc in range(NSC - 1):
            full[c * BS + sc * SCB + map_rp.reshape(-1), 0] = rr[:, sc, :].reshape(-1)
        rc = np.asarray(results[c]["outc"], np.float32)
        full[c * BS + (NSC - 1) * SCB + map_cs.reshape(-1), 0] = rc.reshape(-1)
    return full


def kernel(x, W_leaf, weights, biases, w_out, b_out, _run_kwargs=None):
    x = np.asarray(x, dtype=np.float32)
    W_leaf = np.asarray(W_leaf, dtype=np.float32)
    weights = np.asarray(weights, dtype=np.float32)
    biases = np.asarray(biases, dtype=np.float32)
    w_out = np.asarray(w_out, dtype=np.float32)
    b_out = np.asarray(b_out, dtype=np.float32)
    in_maps, gscale = make_in_maps(x, W_leaf, weights, biases, w_out, b_out)
    nc = build_nc(gscale)
    kw = dict(_run_kwargs or {})
    res = run_bass_kernel_spmd(nc, in_maps, core_ids=list(range(N_CORES)), **kw)
    out = gather_out(res.results)
    if _run_kwargs is not None:
        kernel.last_results = res
    return out
